# revision 13
# baseline (speedup 1.0000x reference)
"""BloomAttention (B=2, S=1024, H=4096, 32 heads, head_dim=128) on 8 TRN2
NeuronCores — tensor-parallel over heads (4 heads per core).

Strategy (per core, SPMD — one Bass program, per-core data):
  * hidden_states are pre-transposed on host to hidT [B, H, S] and cast to
    bf16 (replicated); w_qkv column-sliced per core, w_out row-sliced, both
    bf16 (halves HBM traffic; PE streams bf16 at the same 1 col/cycle).
  * QKV: chunked-K GEMMs (4 chunks x 8 K-tiles), fp32 PSUM chains, fp32
    SBUF accumulation across chunks, with the final chunk's eviction fused
    with the bf16 downcast of qT/kT/v (zero extra passes).
  * Attention per head, 256-col q panels (finer causal granularity: 20 of
    32 (k,q) tile-pairs instead of 24):
      scoresT[k,q] = kT.T @ qT  (bf16, k on partitions)
      P = exp(scoresT) [Scalar] * EAD [GpSimd]   (EAD = exp(slope*(j-i))
      with the causal mask baked in as exact zeros)
      ctxT += v.T @ P ; den += 1s.T @ P   (both PSUM-accumulated; ctx/den
      for a head pair packed into shared PSUM banks)
      ctx = ctxT * reciprocal_approx_fast(den)  [DVE]
  * Projection (w_out resident in SBUF, loaded once) interleaved per
    512-col block as soon as its ctx tiles complete; bf16 partials
    [H, B*S] written out; the host sums the 8 partials (the "all-reduce"),
    transposes back, and adds b_out + b_v @ w_out (v-bias commutes
    through the softmax-linear ops).
  * q/k biases applied on-chip during PSUM eviction; attention scaling
    folded into the q eviction.
"""

import math
import numpy as np
from contextlib import ExitStack

import concourse.bass as bass
import concourse.tile as tile
import concourse.mybir as mybir
from concourse import bacc
from concourse.bass_utils import run_bass_kernel_spmd

f32 = mybir.dt.float32
f32r = mybir.dt.float32r
bf16 = mybir.dt.bfloat16
AF = mybir.ActivationFunctionType
ALU = mybir.AluOpType

B, S, H = 2, 1024, 4096
TOTAL_HEADS = 32
N_CORES = 8
HPC = TOTAL_HEADS // N_CORES      # heads per core
HD = HPC * 128                    # per-core head feature width
OFF = 384                         # D-table offset
W = OFF + S                       # D-table width
MASK_FILL = -1.0e5
N_CHUNKS = 4


def _build_nc(n_devices=N_CORES, repeat=1, nonce=1):
    hpc = HPC
    NH_T = H // 128               # 32 K-tiles over hidden dim
    CH = NH_T // N_CHUNKS         # 8 K-tiles per chunk
    SB = S // 512                 # 2 proj/QKV seq blocks
    ST = S // 128                 # 8 k-tiles over seq
    QC = S // 256                 # 4 attention q panels
    OG = H // 512                 # 8 proj output groups
    scaling = float(128 ** -0.5)

    nc = bacc.Bacc("TRN2", target_bir_lowering=False, debug=False,
                   num_devices=n_devices)
    hidT = nc.dram_tensor("hidT", [B, H, S], bf16, kind="ExternalInput").ap()
    wq = nc.dram_tensor("wq", [H, HD], bf16, kind="ExternalInput").ap()
    wk = nc.dram_tensor("wk", [H, HD], bf16, kind="ExternalInput").ap()
    wv = nc.dram_tensor("wv", [H, HD], bf16, kind="ExternalInput").ap()
    wo = nc.dram_tensor("wo", [HD, H], bf16, kind="ExternalInput").ap()
    bq = nc.dram_tensor("bq", [128, hpc], f32, kind="ExternalInput").ap()
    bk = nc.dram_tensor("bk", [128, hpc], f32, kind="ExternalInput").ap()
    slp = nc.dram_tensor("slp", [128, hpc], f32, kind="ExternalInput").ap()
    outp = nc.dram_tensor("outp", [H, B * S], bf16, kind="ExternalOutput").ap()
    nonce_t = nc.dram_tensor("nonce", [1, int(nonce)], f32,
                             kind="ExternalInput").ap()
    del nonce_t

    with tile.TileContext(nc) as tc:
        with ExitStack() as ctx:
            const = ctx.enter_context(tc.tile_pool(name="const", bufs=1))
            hidp = ctx.enter_context(tc.tile_pool(name="hidp", bufs=CH + CH // 2))
            wsp = ctx.enter_context(tc.tile_pool(name="wsp", bufs=2 * CH + 2))
            accp = ctx.enter_context(tc.tile_pool(name="accp", bufs=8))
            qkp = ctx.enter_context(tc.tile_pool(name="qkp", bufs=2 * hpc))
            vnp = ctx.enter_context(tc.tile_pool(name="vnp", bufs=ST))
            pp = ctx.enter_context(tc.tile_pool(name="pp", bufs=4))
            rp = ctx.enter_context(tc.tile_pool(name="rp", bufs=4))
            ctxp = ctx.enter_context(tc.tile_pool(name="ctxp", bufs=hpc * SB))
            osp = ctx.enter_context(tc.tile_pool(name="osp", bufs=6))
            psp = ctx.enter_context(tc.tile_pool(name="psp", bufs=4, space="PSUM"))

            ps_ctr = [0]

            def flow_tile(cols=512):
                ps_ctr[0] += 1
                return psp.tile([128, cols], f32, tag="flow", bufs=4,
                                name=f"psf_{ps_ctr[0]}")

            def held_tile():
                ps_ctr[0] += 1
                return psp.tile([128, 512], f32, tag="held", bufs=4,
                                name=f"psh_{ps_ctr[0]}")

            # ---- constants ----
            Dext = const.tile([128, W], f32, tag="dext")
            nc.gpsimd.iota(Dext[:], base=OFF, channel_multiplier=1,
                           pattern=[[-1, W]],
                           allow_small_or_imprecise_dtypes=True)
            nc.gpsimd.affine_select(Dext[:], Dext[:], base=-OFF,
                                    channel_multiplier=-1, pattern=[[1, W]],
                                    compare_op=ALU.is_ge, fill=MASK_FILL)
            ones_f = const.tile([128, 128], f32, tag="onesf")
            nc.gpsimd.memset(ones_f[:], 1.0)
            ones = const.tile([128, 128], bf16, tag="ones")
            nc.vector.tensor_copy(ones[:], ones_f[:])
            bq_t = const.tile([128, hpc], f32, tag="bq")
            nc.sync.dma_start(bq_t[:], bq[:])
            bk_t = const.tile([128, hpc], f32, tag="bk")
            nc.sync.dma_start(bk_t[:], bk[:])
            slp_t = const.tile([128, hpc], f32, tag="slp")
            nc.sync.dma_start(slp_t[:], slp[:])
            # alibi tables, one per head (batch-independent)
            EAD = []
            for head in range(hpc):
                ead = const.tile([128, W], bf16, tag="ead", bufs=hpc,
                                 name=f"ead_{head}")
                nc.scalar.activation(ead[:], Dext[:], AF.Exp,
                                     scale=slp_t[:, head:head + 1])
                EAD.append(ead)
            wo_t = {}

            for bi in range(B * repeat):
                b = bi % B
                # ================= QKV =================
                qk_final = {
                    "q": [qkp.tile([128, S], bf16, tag="qkT",
                                   name=f"qT_{bi}_{i}") for i in range(hpc)],
                    "k": [qkp.tile([128, S], bf16, tag="qkT",
                                   name=f"kT_{bi}_{i}") for i in range(hpc)],
                }
                v_final = [vnp.tile([128, HD], bf16, tag="vn",
                                    name=f"vN_{bi}_{i}") for i in range(ST)]
                qk_acc = {
                    "q": [accp.tile([128, S], f32, tag="qkacc", bufs=8,
                                    name=f"qA_{bi}_{i}") for i in range(hpc)],
                    "k": [accp.tile([128, S], f32, tag="qkacc", bufs=8,
                                    name=f"kA_{bi}_{i}") for i in range(hpc)],
                }
                v_acc = [accp.tile([128, HD], f32, tag="vacc", bufs=ST,
                                   name=f"vA_{bi}_{i}") for i in range(ST)]

                for hc in range(N_CHUNKS):
                    hts = list(range(hc * CH, (hc + 1) * CH))
                    hid_t = {}
                    w_t = {}
                    for ht in hts:
                        t = hidp.tile([128, S], bf16, tag="hidt",
                                      name=f"hid_{bi}_{ht}")
                        nc.sync.dma_start(
                            t[:], hidT[b, ht * 128:(ht + 1) * 128, :])
                        hid_t[ht] = t
                        wqt = wsp.tile([128, HD], bf16, tag="w",
                                       name=f"wq_{bi}_{ht}")
                        nc.sync.dma_start(
                            wqt[:], wq[ht * 128:(ht + 1) * 128, :])
                        w_t[("q", ht)] = wqt

                    # Q then K: output-stationary chains per (head, sub)
                    for which, wsrc, bias_t, sc in (
                        ("q", wq, bq_t, scaling), ("k", wk, bk_t, 1.0)):
                        if which == "k":
                            for ht in hts:
                                wkt = wsp.tile([128, HD], bf16, tag="w",
                                               name=f"wk_{bi}_{ht}")
                                nc.sync.dma_start(
                                    wkt[:],
                                    wsrc[ht * 128:(ht + 1) * 128, :])
                                w_t[("k", ht)] = wkt
                        for head in range(hpc):
                            for sub in range(SB):
                                ps = flow_tile()
                                for i, ht in enumerate(hts):
                                    nc.tensor.matmul(
                                        ps[:],
                                        w_t[(which, ht)][:, head * 128:(head + 1) * 128],
                                        hid_t[ht][:, sub * 512:(sub + 1) * 512],
                                        start=(i == 0), stop=(i == CH - 1))
                                acc = qk_acc[which][head][:, sub * 512:(sub + 1) * 512]
                                if hc == 0:
                                    nc.vector.tensor_scalar(
                                        out=acc, in0=ps[:],
                                        scalar1=sc, scalar2=bias_t[:, head:head + 1],
                                        op0=ALU.mult, op1=ALU.add)
                                elif hc < N_CHUNKS - 1:
                                    nc.vector.scalar_tensor_tensor(
                                        out=acc, in0=ps[:],
                                        scalar=sc, in1=acc,
                                        op0=ALU.mult, op1=ALU.add)
                                else:
                                    fin = qk_final[which][head]
                                    nc.vector.scalar_tensor_tensor(
                                        out=fin[:, sub * 512:(sub + 1) * 512],
                                        in0=ps[:], scalar=sc, in1=acc,
                                        op0=ALU.mult, op1=ALU.add)

                    # V: output-stationary chains per s-tile
                    for ht in hts:
                        wvt = wsp.tile([128, HD], bf16, tag="w",
                                       name=f"wv_{bi}_{ht}")
                        nc.sync.dma_start(
                            wvt[:], wv[ht * 128:(ht + 1) * 128, :])
                        w_t[("v", ht)] = wvt
                    for st in range(ST):
                        ps = flow_tile(HD)
                        for i, ht in enumerate(hts):
                            nc.tensor.matmul(
                                ps[:],
                                hid_t[ht][:, st * 128:(st + 1) * 128],
                                w_t[("v", ht)][:],
                                start=(i == 0), stop=(i == CH - 1))
                        if hc == 0:
                            nc.vector.tensor_copy(v_acc[st][:], ps[:])
                        elif hc < N_CHUNKS - 1:
                            nc.vector.tensor_add(v_acc[st][:], ps[:], v_acc[st][:])
                        else:
                            nc.vector.tensor_add(v_final[st][:], ps[:], v_acc[st][:])

                # w_out resident load, once (overlaps batch-0 attention)
                if bi == 0:
                    for og in range(OG):
                        for f in range(hpc):
                            t = const.tile([128, 512], bf16, tag="wo",
                                           bufs=OG * hpc, name=f"wo_{og}_{f}")
                            nc.sync.dma_start(
                                t[:],
                                wo[f * 128:(f + 1) * 128,
                                   og * 512:(og + 1) * 512])
                            wo_t[(og, f)] = t

                # ====== attention (256-col q panels) + interleaved proj ======
                ctx_tiles = {h: [ctxp.tile([128, 512], bf16, tag="ctx",
                                           name=f"ctx_{bi}_{h}_{sb}")
                                 for sb in range(SB)] for h in range(hpc)}
                for qc in range(QC):
                    q0 = qc * 256
                    J = 2 * qc + 2
                    for hp in range(0, hpc, 2):
                        pair = [hp, hp + 1]
                        ps_ctx = held_tile()
                        ps_den = held_tile()
                        cs = {pair[0]: ps_ctx[:, 0:256],
                              pair[1]: ps_ctx[:, 256:512]}
                        ds = {pair[0]: ps_den[:, 0:256],
                              pair[1]: ps_den[:, 256:512]}
                        for j in range(J):
                            w0 = q0 - j * 128 + OFF
                            for h in pair:
                                qT = qk_final["q"][h]
                                kT = qk_final["k"][h]
                                ps_s = flow_tile(256)
                                nc.tensor.matmul(
                                    ps_s[:], kT[:, j * 128:(j + 1) * 128],
                                    qT[:, q0:q0 + 256],
                                    start=True, stop=True)
                                P = pp.tile([128, 256], bf16, tag="P")
                                nc.scalar.activation(P[:], ps_s[:], AF.Exp)
                                nc.gpsimd.tensor_mul(P[:], P[:],
                                                     EAD[h][:, w0:w0 + 256])
                                # start=True clears has_written for the WHOLE
                                # bank, so only the first chain into a shared
                                # bank may use it; the partner head's first
                                # matmul lands on cleared bits and overwrites.
                                first = (j == 0 and h == pair[0])
                                nc.tensor.matmul(
                                    cs[h],
                                    v_final[j][:, h * 128:(h + 1) * 128],
                                    P[:], start=first, stop=(j == J - 1),
                                    skip_group_check=True)
                                nc.tensor.matmul(
                                    ds[h], ones[:], P[:],
                                    start=first, stop=(j == J - 1),
                                    skip_group_check=True)
                        for h in pair:
                            recip = rp.tile([128, 256], f32, tag="recip")
                            nc.vector.reciprocal_approx_fast(recip[:], ds[h])
                            half = (qc % 2) * 256
                            nc.vector.tensor_mul(
                                ctx_tiles[h][qc // 2][:, half:half + 256],
                                cs[h], recip[:])

                    # proj for seq block sb as soon as its ctx completes
                    if qc % 2 == 1:
                        sb = qc // 2
                        for og in range(OG):
                            for ot in range(4):
                                ps_o = flow_tile()
                                for f in range(hpc):
                                    nc.tensor.matmul(
                                        ps_o[:],
                                        wo_t[(og, f)][:, ot * 128:(ot + 1) * 128],
                                        ctx_tiles[f][sb][:],
                                        start=(f == 0), stop=(f == hpc - 1))
                                ost = osp.tile([128, 512], bf16, tag="ost")
                                nc.vector.tensor_copy(ost[:], ps_o[:])
                                r0 = og * 512 + ot * 128
                                c0 = b * S + sb * 512
                                nc.sync.dma_start(
                                    outp[r0:r0 + 128, c0:c0 + 512], ost[:])

    nc.compile()
    return nc


def _alibi_slopes(total_heads):
    closest = 2 ** math.floor(math.log2(total_heads))
    base = 2 ** (-(2 ** (-(math.log2(closest) - 3))))
    powers = np.arange(1, 1 + closest, dtype=np.float32)
    slopes = np.power(base, powers).astype(np.float32)
    if closest != total_heads:
        extra_base = 2 ** (-(2 ** (-(math.log2(2 * closest) - 3))))
        num_rem = min(closest, total_heads - closest)
        extra = np.arange(1, 1 + 2 * num_rem, 2, dtype=np.float32)
        slopes = np.concatenate(
            [slopes, np.power(extra_base, extra).astype(np.float32)])
    return slopes


_NC_CACHE = {}


def _get_nc():
    if "nc" not in _NC_CACHE:
        _NC_CACHE["nc"] = _build_nc()
    return _NC_CACHE["nc"]


def make_in_maps(hidden_states, w_qkv, b_qkv, w_out):
    """Build the 8 per-core input dicts."""
    import ml_dtypes
    bf = ml_dtypes.bfloat16
    scaling = np.float32(128 ** -0.5)
    hidT = np.ascontiguousarray(
        hidden_states.transpose(0, 2, 1)).astype(bf)
    slopes = _alibi_slopes(TOTAL_HEADS)
    nonce = np.zeros((1, 1), np.float32)
    in_maps = []
    for core in range(N_CORES):
        c0 = core * HD
        bq_v = b_qkv[c0:c0 + HD]
        bk_v = b_qkv[H + c0:H + c0 + HD]
        sl = slopes[core * HPC:(core + 1) * HPC]
        in_maps.append(dict(
            hidT=hidT,
            wq=np.ascontiguousarray(w_qkv[:, c0:c0 + HD]).astype(bf),
            wk=np.ascontiguousarray(w_qkv[:, H + c0:H + c0 + HD]).astype(bf),
            wv=np.ascontiguousarray(w_qkv[:, 2 * H + c0:2 * H + c0 + HD]).astype(bf),
            wo=np.ascontiguousarray(w_out[c0:c0 + HD, :]).astype(bf),
            bq=np.ascontiguousarray(
                (bq_v.reshape(HPC, 128).T * scaling).astype(np.float32)),
            bk=np.ascontiguousarray(bk_v.reshape(HPC, 128).T.astype(np.float32)),
            slp=np.ascontiguousarray(
                np.broadcast_to(sl[None, :], (128, HPC)).astype(np.float32)),
            nonce=nonce,
        ))
    return in_maps


def finish_output(partials, b_qkv, w_out, b_out):
    """Host-side all-reduce over cores + layout fix + bias."""
    total = np.zeros((H, B * S), dtype=np.float64)
    for p in partials:
        total += p.astype(np.float64)
    bias_vec = (b_qkv[2 * H:].astype(np.float64) @ w_out.astype(np.float64)
                + b_out.astype(np.float64))
    out = total.reshape(H, B, S).transpose(1, 2, 0) + bias_vec[None, None, :]
    return np.ascontiguousarray(out.astype(np.float32))


def kernel(hidden_states, w_qkv, b_qkv, w_out, b_out):
    hidden_states = np.asarray(hidden_states, dtype=np.float32)
    w_qkv = np.asarray(w_qkv, dtype=np.float32)
    b_qkv = np.asarray(b_qkv, dtype=np.float32)
    w_out = np.asarray(w_out, dtype=np.float32)
    b_out = np.asarray(b_out, dtype=np.float32)

    nc = _get_nc()
    in_maps = make_in_maps(hidden_states, w_qkv, b_qkv, w_out)
    res = run_bass_kernel_spmd(nc, in_maps, core_ids=list(range(N_CORES)))
    return finish_output([res.results[c]["outp"] for c in range(N_CORES)],
                         b_qkv, w_out, b_out)


# revision 14
# speedup vs baseline: 1.0198x; 1.0198x over previous
"""BloomAttention (B=2, S=1024, H=4096, 32 heads, head_dim=128) on 8 TRN2
NeuronCores — tensor-parallel over heads (4 heads per core).

Strategy (per core, SPMD — one Bass program, per-core data):
  * hidden_states are pre-transposed on host to hidT [B, H, S] and cast to
    bf16 (replicated); w_qkv column-sliced per core, w_out row-sliced, both
    bf16 (halves HBM traffic; PE streams bf16 at the same 1 col/cycle).
  * QKV: chunked-K GEMMs (4 chunks x 8 K-tiles), fp32 PSUM chains, fp32
    SBUF accumulation across chunks, with the final chunk's eviction fused
    with the bf16 downcast of qT/kT/v (zero extra passes).
  * Attention per head, 256-col q panels (finer causal granularity: 20 of
    32 (k,q) tile-pairs instead of 24):
      scoresT[k,q] = kT.T @ qT  (bf16, k on partitions)
      P = exp(scoresT) [Scalar] * EAD [GpSimd]   (EAD = exp(slope*(j-i))
      with the causal mask baked in as exact zeros)
      ctxT += v.T @ P ; den += 1s.T @ P   (both PSUM-accumulated; ctx/den
      for a head pair packed into shared PSUM banks)
      ctx = ctxT * reciprocal_approx_fast(den)  [DVE]
  * Projection (w_out resident in SBUF, loaded once) interleaved per
    512-col block as soon as its ctx tiles complete; bf16 partials
    [H, B*S] written out; the host sums the 8 partials (the "all-reduce"),
    transposes back, and adds b_out + b_v @ w_out (v-bias commutes
    through the softmax-linear ops).
  * q/k biases applied on-chip during PSUM eviction; attention scaling
    folded into the q eviction.
"""

import math
import numpy as np
from contextlib import ExitStack

import concourse.bass as bass
import concourse.tile as tile
import concourse.mybir as mybir
from concourse import bacc
from concourse.bass_utils import run_bass_kernel_spmd

f32 = mybir.dt.float32
f32r = mybir.dt.float32r
bf16 = mybir.dt.bfloat16
AF = mybir.ActivationFunctionType
ALU = mybir.AluOpType

B, S, H = 2, 1024, 4096
TOTAL_HEADS = 32
N_CORES = 8
HPC = TOTAL_HEADS // N_CORES      # heads per core
HD = HPC * 128                    # per-core head feature width
OFF = 384                         # D-table offset
W = OFF + S                       # D-table width
MASK_FILL = -1.0e5
N_CHUNKS = 4


def _build_nc(n_devices=N_CORES, repeat=1, nonce=1):
    hpc = HPC
    NH_T = H // 128               # 32 K-tiles over hidden dim
    CH = NH_T // N_CHUNKS         # 8 K-tiles per chunk
    SB = S // 512                 # 2 proj/QKV seq blocks
    ST = S // 128                 # 8 k-tiles over seq
    QC = S // 256                 # 4 attention q panels
    OG = H // 512                 # 8 proj output groups
    scaling = float(128 ** -0.5)

    nc = bacc.Bacc("TRN2", target_bir_lowering=False, debug=False,
                   num_devices=n_devices)
    hidT = nc.dram_tensor("hidT", [B, H, S], bf16, kind="ExternalInput").ap()
    wq = nc.dram_tensor("wq", [H, HD], bf16, kind="ExternalInput").ap()
    wk = nc.dram_tensor("wk", [H, HD], bf16, kind="ExternalInput").ap()
    wv = nc.dram_tensor("wv", [H, HD], bf16, kind="ExternalInput").ap()
    wo = nc.dram_tensor("wo", [HD, H], bf16, kind="ExternalInput").ap()
    bq = nc.dram_tensor("bq", [128, hpc], f32, kind="ExternalInput").ap()
    bk = nc.dram_tensor("bk", [128, hpc], f32, kind="ExternalInput").ap()
    slp = nc.dram_tensor("slp", [128, hpc], f32, kind="ExternalInput").ap()
    outp = nc.dram_tensor("outp", [H, B * S], bf16, kind="ExternalOutput").ap()
    nonce_t = nc.dram_tensor("nonce", [1, int(nonce)], f32,
                             kind="ExternalInput").ap()
    del nonce_t

    with tile.TileContext(nc) as tc:
        with ExitStack() as ctx:
            const = ctx.enter_context(tc.tile_pool(name="const", bufs=1))
            hidp = ctx.enter_context(tc.tile_pool(name="hidp", bufs=CH + CH // 2))
            wsp = ctx.enter_context(tc.tile_pool(name="wsp", bufs=2 * CH + 2))
            accp = ctx.enter_context(tc.tile_pool(name="accp", bufs=8))
            qkp = ctx.enter_context(tc.tile_pool(name="qkp", bufs=2 * hpc))
            vnp = ctx.enter_context(tc.tile_pool(name="vnp", bufs=ST))
            pp = ctx.enter_context(tc.tile_pool(name="pp", bufs=4))
            rp = ctx.enter_context(tc.tile_pool(name="rp", bufs=4))
            ctxp = ctx.enter_context(tc.tile_pool(name="ctxp", bufs=hpc * SB))
            osp = ctx.enter_context(tc.tile_pool(name="osp", bufs=6))
            psp = ctx.enter_context(tc.tile_pool(name="psp", bufs=4, space="PSUM"))

            ps_ctr = [0]

            def flow_tile(cols=512):
                ps_ctr[0] += 1
                return psp.tile([128, cols], f32, tag="flow", bufs=4,
                                name=f"psf_{ps_ctr[0]}")

            def held_tile():
                ps_ctr[0] += 1
                return psp.tile([128, 512], f32, tag="held", bufs=4,
                                name=f"psh_{ps_ctr[0]}")

            # ---- constants ----
            Dext = const.tile([128, W], f32, tag="dext")
            nc.gpsimd.iota(Dext[:], base=OFF, channel_multiplier=1,
                           pattern=[[-1, W]],
                           allow_small_or_imprecise_dtypes=True)
            nc.gpsimd.affine_select(Dext[:], Dext[:], base=-OFF,
                                    channel_multiplier=-1, pattern=[[1, W]],
                                    compare_op=ALU.is_ge, fill=MASK_FILL)
            ones_f = const.tile([128, 128], f32, tag="onesf")
            nc.gpsimd.memset(ones_f[:], 1.0)
            ones = const.tile([128, 128], bf16, tag="ones")
            nc.vector.tensor_copy(ones[:], ones_f[:])
            bq_t = const.tile([128, hpc], f32, tag="bq")
            nc.sync.dma_start(bq_t[:], bq[:])
            bk_t = const.tile([128, hpc], f32, tag="bk")
            nc.sync.dma_start(bk_t[:], bk[:])
            slp_t = const.tile([128, hpc], f32, tag="slp")
            nc.sync.dma_start(slp_t[:], slp[:])
            # alibi tables, one per head (batch-independent)
            EAD = []
            for head in range(hpc):
                ead = const.tile([128, W], bf16, tag="ead", bufs=hpc,
                                 name=f"ead_{head}")
                nc.scalar.activation(ead[:], Dext[:], AF.Exp,
                                     scale=slp_t[:, head:head + 1])
                EAD.append(ead)
            wo_t = {}

            for bi in range(B * repeat):
                b = bi % B
                # ================= QKV =================
                qk_final = {
                    "q": [qkp.tile([128, S], bf16, tag="qkT",
                                   name=f"qT_{bi}_{i}") for i in range(hpc)],
                    "k": [qkp.tile([128, S], bf16, tag="qkT",
                                   name=f"kT_{bi}_{i}") for i in range(hpc)],
                }
                v_final = [vnp.tile([128, HD], bf16, tag="vn",
                                    name=f"vN_{bi}_{i}") for i in range(ST)]
                qk_acc = {
                    "q": [accp.tile([128, S], f32, tag="qkacc", bufs=8,
                                    name=f"qA_{bi}_{i}") for i in range(hpc)],
                    "k": [accp.tile([128, S], f32, tag="qkacc", bufs=8,
                                    name=f"kA_{bi}_{i}") for i in range(hpc)],
                }
                v_acc = [accp.tile([128, HD], f32, tag="vacc", bufs=ST,
                                   name=f"vA_{bi}_{i}") for i in range(ST)]

                for hc in range(N_CHUNKS):
                    hts = list(range(hc * CH, (hc + 1) * CH))
                    hid_t = {}
                    w_t = {}
                    for ht in hts:
                        t = hidp.tile([128, S], bf16, tag="hidt",
                                      name=f"hid_{bi}_{ht}")
                        nc.sync.dma_start(
                            t[:], hidT[b, ht * 128:(ht + 1) * 128, :])
                        hid_t[ht] = t
                        wqt = wsp.tile([128, HD], bf16, tag="w",
                                       name=f"wq_{bi}_{ht}")
                        nc.sync.dma_start(
                            wqt[:], wq[ht * 128:(ht + 1) * 128, :])
                        w_t[("q", ht)] = wqt

                    # Q then K: output-stationary chains per (head, sub)
                    for which, wsrc, bias_t, sc in (
                        ("q", wq, bq_t, scaling), ("k", wk, bk_t, 1.0)):
                        if which == "k":
                            for ht in hts:
                                wkt = wsp.tile([128, HD], bf16, tag="w",
                                               name=f"wk_{bi}_{ht}")
                                nc.sync.dma_start(
                                    wkt[:],
                                    wsrc[ht * 128:(ht + 1) * 128, :])
                                w_t[("k", ht)] = wkt
                        for head in range(hpc):
                            for sub in range(SB):
                                ps = flow_tile()
                                for i, ht in enumerate(hts):
                                    nc.tensor.matmul(
                                        ps[:],
                                        w_t[(which, ht)][:, head * 128:(head + 1) * 128],
                                        hid_t[ht][:, sub * 512:(sub + 1) * 512],
                                        start=(i == 0), stop=(i == CH - 1))
                                acc = qk_acc[which][head][:, sub * 512:(sub + 1) * 512]
                                if hc == 0:
                                    nc.vector.tensor_scalar(
                                        out=acc, in0=ps[:],
                                        scalar1=sc, scalar2=bias_t[:, head:head + 1],
                                        op0=ALU.mult, op1=ALU.add)
                                elif hc < N_CHUNKS - 1:
                                    nc.vector.scalar_tensor_tensor(
                                        out=acc, in0=ps[:],
                                        scalar=sc, in1=acc,
                                        op0=ALU.mult, op1=ALU.add)
                                else:
                                    fin = qk_final[which][head]
                                    nc.vector.scalar_tensor_tensor(
                                        out=fin[:, sub * 512:(sub + 1) * 512],
                                        in0=ps[:], scalar=sc, in1=acc,
                                        op0=ALU.mult, op1=ALU.add)

                    # V: output-stationary chains per s-tile
                    for ht in hts:
                        wvt = wsp.tile([128, HD], bf16, tag="w",
                                       name=f"wv_{bi}_{ht}")
                        nc.sync.dma_start(
                            wvt[:], wv[ht * 128:(ht + 1) * 128, :])
                        w_t[("v", ht)] = wvt
                    for st in range(ST):
                        ps = flow_tile(HD)
                        for i, ht in enumerate(hts):
                            nc.tensor.matmul(
                                ps[:],
                                hid_t[ht][:, st * 128:(st + 1) * 128],
                                w_t[("v", ht)][:],
                                start=(i == 0), stop=(i == CH - 1))
                        if hc == 0:
                            nc.vector.tensor_copy(v_acc[st][:], ps[:])
                        elif hc < N_CHUNKS - 1:
                            nc.vector.tensor_add(v_acc[st][:], ps[:], v_acc[st][:])
                        else:
                            nc.vector.tensor_add(v_final[st][:], ps[:], v_acc[st][:])

                # w_out resident load, once (overlaps batch-0 attention)
                if bi == 0:
                    for og in range(OG):
                        for f in range(hpc):
                            t = const.tile([128, 512], bf16, tag="wo",
                                           bufs=OG * hpc, name=f"wo_{og}_{f}")
                            nc.sync.dma_start(
                                t[:],
                                wo[f * 128:(f + 1) * 128,
                                   og * 512:(og + 1) * 512])
                            wo_t[(og, f)] = t

                # ====== attention (256-col q panels) + interleaved proj ======
                ctx_tiles = {h: [ctxp.tile([128, 512], bf16, tag="ctx",
                                           name=f"ctx_{bi}_{h}_{sb}")
                                 for sb in range(SB)] for h in range(hpc)}
                for qc in range(QC):
                    q0 = qc * 256
                    J = 2 * qc + 2
                    for hp in range(0, hpc, 2):
                        pair = [hp, hp + 1]
                        ps_ctx = held_tile()
                        ps_den = held_tile()
                        cs = {pair[0]: ps_ctx[:, 0:256],
                              pair[1]: ps_ctx[:, 256:512]}
                        ds = {pair[0]: ps_den[:, 0:256],
                              pair[1]: ps_den[:, 256:512]}
                        for j in range(J):
                            w0 = q0 - j * 128 + OFF
                            for h in pair:
                                qT = qk_final["q"][h]
                                kT = qk_final["k"][h]
                                ps_s = flow_tile(256)
                                nc.tensor.matmul(
                                    ps_s[:], kT[:, j * 128:(j + 1) * 128],
                                    qT[:, q0:q0 + 256],
                                    start=True, stop=True)
                                P = pp.tile([128, 256], bf16, tag="P")
                                nc.scalar.activation(P[:], ps_s[:], AF.Exp)
                                nc.vector.tensor_mul(P[:], P[:],
                                                     EAD[h][:, w0:w0 + 256])
                                # start=True clears has_written for the WHOLE
                                # bank, so only the first chain into a shared
                                # bank may use it; the partner head's first
                                # matmul lands on cleared bits and overwrites.
                                first = (j == 0 and h == pair[0])
                                nc.tensor.matmul(
                                    cs[h],
                                    v_final[j][:, h * 128:(h + 1) * 128],
                                    P[:], start=first, stop=(j == J - 1),
                                    skip_group_check=True)
                                nc.tensor.matmul(
                                    ds[h], ones[:], P[:],
                                    start=first, stop=(j == J - 1),
                                    skip_group_check=True)
                        for h in pair:
                            recip = rp.tile([128, 256], f32, tag="recip")
                            nc.vector.reciprocal_approx_fast(recip[:], ds[h])
                            half = (qc % 2) * 256
                            nc.vector.tensor_mul(
                                ctx_tiles[h][qc // 2][:, half:half + 256],
                                cs[h], recip[:])

                    # proj for seq block sb as soon as its ctx completes
                    if qc % 2 == 1:
                        sb = qc // 2
                        for og in range(OG):
                            for ot in range(4):
                                ps_o = flow_tile()
                                for f in range(hpc):
                                    nc.tensor.matmul(
                                        ps_o[:],
                                        wo_t[(og, f)][:, ot * 128:(ot + 1) * 128],
                                        ctx_tiles[f][sb][:],
                                        start=(f == 0), stop=(f == hpc - 1))
                                ost = osp.tile([128, 512], bf16, tag="ost")
                                nc.vector.tensor_copy(ost[:], ps_o[:])
                                r0 = og * 512 + ot * 128
                                c0 = b * S + sb * 512
                                nc.sync.dma_start(
                                    outp[r0:r0 + 128, c0:c0 + 512], ost[:])

    nc.compile()
    return nc


def _alibi_slopes(total_heads):
    closest = 2 ** math.floor(math.log2(total_heads))
    base = 2 ** (-(2 ** (-(math.log2(closest) - 3))))
    powers = np.arange(1, 1 + closest, dtype=np.float32)
    slopes = np.power(base, powers).astype(np.float32)
    if closest != total_heads:
        extra_base = 2 ** (-(2 ** (-(math.log2(2 * closest) - 3))))
        num_rem = min(closest, total_heads - closest)
        extra = np.arange(1, 1 + 2 * num_rem, 2, dtype=np.float32)
        slopes = np.concatenate(
            [slopes, np.power(extra_base, extra).astype(np.float32)])
    return slopes


_NC_CACHE = {}


def _get_nc():
    if "nc" not in _NC_CACHE:
        _NC_CACHE["nc"] = _build_nc()
    return _NC_CACHE["nc"]


def make_in_maps(hidden_states, w_qkv, b_qkv, w_out):
    """Build the 8 per-core input dicts."""
    import ml_dtypes
    bf = ml_dtypes.bfloat16
    scaling = np.float32(128 ** -0.5)
    hidT = np.ascontiguousarray(
        hidden_states.transpose(0, 2, 1)).astype(bf)
    slopes = _alibi_slopes(TOTAL_HEADS)
    nonce = np.zeros((1, 1), np.float32)
    in_maps = []
    for core in range(N_CORES):
        c0 = core * HD
        bq_v = b_qkv[c0:c0 + HD]
        bk_v = b_qkv[H + c0:H + c0 + HD]
        sl = slopes[core * HPC:(core + 1) * HPC]
        in_maps.append(dict(
            hidT=hidT,
            wq=np.ascontiguousarray(w_qkv[:, c0:c0 + HD]).astype(bf),
            wk=np.ascontiguousarray(w_qkv[:, H + c0:H + c0 + HD]).astype(bf),
            wv=np.ascontiguousarray(w_qkv[:, 2 * H + c0:2 * H + c0 + HD]).astype(bf),
            wo=np.ascontiguousarray(w_out[c0:c0 + HD, :]).astype(bf),
            bq=np.ascontiguousarray(
                (bq_v.reshape(HPC, 128).T * scaling).astype(np.float32)),
            bk=np.ascontiguousarray(bk_v.reshape(HPC, 128).T.astype(np.float32)),
            slp=np.ascontiguousarray(
                np.broadcast_to(sl[None, :], (128, HPC)).astype(np.float32)),
            nonce=nonce,
        ))
    return in_maps


def finish_output(partials, b_qkv, w_out, b_out):
    """Host-side all-reduce over cores + layout fix + bias."""
    total = np.zeros((H, B * S), dtype=np.float64)
    for p in partials:
        total += p.astype(np.float64)
    bias_vec = (b_qkv[2 * H:].astype(np.float64) @ w_out.astype(np.float64)
                + b_out.astype(np.float64))
    out = total.reshape(H, B, S).transpose(1, 2, 0) + bias_vec[None, None, :]
    return np.ascontiguousarray(out.astype(np.float32))


def kernel(hidden_states, w_qkv, b_qkv, w_out, b_out):
    hidden_states = np.asarray(hidden_states, dtype=np.float32)
    w_qkv = np.asarray(w_qkv, dtype=np.float32)
    b_qkv = np.asarray(b_qkv, dtype=np.float32)
    w_out = np.asarray(w_out, dtype=np.float32)
    b_out = np.asarray(b_out, dtype=np.float32)

    nc = _get_nc()
    in_maps = make_in_maps(hidden_states, w_qkv, b_qkv, w_out)
    res = run_bass_kernel_spmd(nc, in_maps, core_ids=list(range(N_CORES)))
    return finish_output([res.results[c]["outp"] for c in range(N_CORES)],
                         b_qkv, w_out, b_out)


# revision 15
# speedup vs baseline: 1.3436x; 1.3176x over previous
"""BloomAttention (B=2, S=1024, H=4096, 32 heads, head_dim=128) on 8 TRN2
NeuronCores — tensor-parallel over heads (4 heads per core).

Strategy (per core, SPMD — one Bass program, per-core data):
  * hidden_states are pre-transposed on host to hidT [B, H, S] and cast to
    bf16 (replicated); w_qkv column-sliced per core, w_out row-sliced, both
    bf16 (halves HBM traffic; PE streams bf16 at the same 1 col/cycle).
  * QKV: chunked-K GEMMs (4 chunks x 8 K-tiles), fp32 PSUM chains, fp32
    SBUF accumulation across chunks, with the final chunk's eviction fused
    with the bf16 downcast of qT/kT/v (zero extra passes).
  * Attention per head, 256-col q panels (finer causal granularity: 20 of
    32 (k,q) tile-pairs instead of 24):
      scoresT[k,q] = kT.T @ qT  (bf16, k on partitions)
      P = exp(scoresT) [Scalar] * EAD [GpSimd]   (EAD = exp(slope*(j-i))
      with the causal mask baked in as exact zeros)
      ctxT += v.T @ P ; den += 1s.T @ P   (both PSUM-accumulated; ctx/den
      for a head pair packed into shared PSUM banks)
      ctx = ctxT * reciprocal_approx_fast(den)  [DVE]
  * Projection (w_out resident in SBUF, loaded once) interleaved per
    512-col block as soon as its ctx tiles complete; bf16 partials
    [H, B*S] written out; the host sums the 8 partials (the "all-reduce"),
    transposes back, and adds b_out + b_v @ w_out (v-bias commutes
    through the softmax-linear ops).
  * q/k biases applied on-chip during PSUM eviction; attention scaling
    folded into the q eviction.
"""

import math
import numpy as np
from contextlib import ExitStack

import concourse.bass as bass
import concourse.tile as tile
import concourse.mybir as mybir
from concourse import bacc
from concourse.bass_utils import run_bass_kernel_spmd

f32 = mybir.dt.float32
f32r = mybir.dt.float32r
bf16 = mybir.dt.bfloat16
AF = mybir.ActivationFunctionType
ALU = mybir.AluOpType

B, S, H = 2, 1024, 4096
TOTAL_HEADS = 32
N_CORES = 8
HPC = TOTAL_HEADS // N_CORES      # heads per core
HD = HPC * 128                    # per-core head feature width
OFF = 384                         # D-table offset
W = OFF + S                       # D-table width
MASK_FILL = -1.0e5
N_CHUNKS = 4


def _build_nc(n_devices=N_CORES, repeat=1, nonce=1):
    hpc = HPC
    NH_T = H // 128               # 32 K-tiles over hidden dim
    CH = NH_T // N_CHUNKS         # 8 K-tiles per chunk
    SB = S // 512                 # 2 proj/QKV seq blocks
    ST = S // 128                 # 8 k-tiles over seq
    QC = S // 256                 # 4 attention q panels
    OG = H // 512                 # 8 proj output groups
    scaling = float(128 ** -0.5)

    nc = bacc.Bacc("TRN2", target_bir_lowering=False, debug=False,
                   num_devices=n_devices)
    hidT = nc.dram_tensor("hidT", [B, H, S], bf16, kind="ExternalInput").ap()
    wq = nc.dram_tensor("wq", [H, HD], bf16, kind="ExternalInput").ap()
    wk = nc.dram_tensor("wk", [H, HD], bf16, kind="ExternalInput").ap()
    wv = nc.dram_tensor("wv", [H, HD], bf16, kind="ExternalInput").ap()
    wo = nc.dram_tensor("wo", [HD, H], bf16, kind="ExternalInput").ap()
    bq = nc.dram_tensor("bq", [128, hpc], f32, kind="ExternalInput").ap()
    bk = nc.dram_tensor("bk", [128, hpc], f32, kind="ExternalInput").ap()
    slp = nc.dram_tensor("slp", [128, hpc], f32, kind="ExternalInput").ap()
    outp = nc.dram_tensor("outp", [H, B * S], bf16, kind="ExternalOutput").ap()
    nonce_t = nc.dram_tensor("nonce", [1, int(nonce)], f32,
                             kind="ExternalInput").ap()
    del nonce_t

    with tile.TileContext(nc) as tc:
        with ExitStack() as ctx:
            const = ctx.enter_context(tc.tile_pool(name="const", bufs=1))
            hidp = ctx.enter_context(tc.tile_pool(name="hidp", bufs=2 * CH))
            wsp = ctx.enter_context(tc.tile_pool(name="wsp", bufs=2 * CH + 6))
            accp = ctx.enter_context(tc.tile_pool(name="accp", bufs=8))
            qkp = ctx.enter_context(tc.tile_pool(name="qkp", bufs=2 * hpc))
            vnp = ctx.enter_context(tc.tile_pool(name="vnp", bufs=ST))
            pp = ctx.enter_context(tc.tile_pool(name="pp", bufs=4))
            rp = ctx.enter_context(tc.tile_pool(name="rp", bufs=4))
            ctxp = ctx.enter_context(tc.tile_pool(name="ctxp", bufs=hpc * SB))
            osp = ctx.enter_context(tc.tile_pool(name="osp", bufs=6))
            psp = ctx.enter_context(tc.tile_pool(name="psp", bufs=4, space="PSUM"))

            ps_ctr = [0]

            def flow_tile(cols=512):
                ps_ctr[0] += 1
                return psp.tile([128, cols], f32, tag="flow", bufs=4,
                                name=f"psf_{ps_ctr[0]}")

            def held_tile():
                ps_ctr[0] += 1
                return psp.tile([128, 512], f32, tag="held", bufs=4,
                                name=f"psh_{ps_ctr[0]}")

            # ---- constants ----
            Dext = const.tile([128, W], f32, tag="dext")
            nc.gpsimd.iota(Dext[:], base=OFF, channel_multiplier=1,
                           pattern=[[-1, W]],
                           allow_small_or_imprecise_dtypes=True)
            nc.gpsimd.affine_select(Dext[:], Dext[:], base=-OFF,
                                    channel_multiplier=-1, pattern=[[1, W]],
                                    compare_op=ALU.is_ge, fill=MASK_FILL)
            ones_f = const.tile([128, 128], f32, tag="onesf")
            nc.gpsimd.memset(ones_f[:], 1.0)
            ones = const.tile([128, 128], bf16, tag="ones")
            nc.vector.tensor_copy(ones[:], ones_f[:])
            bq_t = const.tile([128, hpc], f32, tag="bq")
            nc.sync.dma_start(bq_t[:], bq[:])
            bk_t = const.tile([128, hpc], f32, tag="bk")
            nc.sync.dma_start(bk_t[:], bk[:])
            slp_t = const.tile([128, hpc], f32, tag="slp")
            nc.sync.dma_start(slp_t[:], slp[:])
            # alibi tables, one per head (batch-independent)
            EAD = []
            for head in range(hpc):
                ead = const.tile([128, W], bf16, tag="ead", bufs=hpc,
                                 name=f"ead_{head}")
                nc.scalar.activation(ead[:], Dext[:], AF.Exp,
                                     scale=slp_t[:, head:head + 1])
                EAD.append(ead)
            wo_t = {}

            for bi in range(B * repeat):
                b = bi % B
                # ================= QKV =================
                qk_final = {
                    "q": [qkp.tile([128, S], bf16, tag="qkT",
                                   name=f"qT_{bi}_{i}") for i in range(hpc)],
                    "k": [qkp.tile([128, S], bf16, tag="qkT",
                                   name=f"kT_{bi}_{i}") for i in range(hpc)],
                }
                v_final = [vnp.tile([128, HD], bf16, tag="vn",
                                    name=f"vN_{bi}_{i}") for i in range(ST)]
                qk_acc = {
                    "q": [accp.tile([128, S], f32, tag="qkacc", bufs=8,
                                    name=f"qA_{bi}_{i}") for i in range(hpc)],
                    "k": [accp.tile([128, S], f32, tag="qkacc", bufs=8,
                                    name=f"kA_{bi}_{i}") for i in range(hpc)],
                }
                v_acc = [accp.tile([128, HD], f32, tag="vacc", bufs=ST,
                                   name=f"vA_{bi}_{i}") for i in range(ST)]

                for hc in range(N_CHUNKS):
                    hts = list(range(hc * CH, (hc + 1) * CH))
                    hid_t = {}
                    w_t = {}
                    for ht in hts:
                        t = hidp.tile([128, S], bf16, tag="hidt",
                                      name=f"hid_{bi}_{ht}")
                        nc.sync.dma_start(
                            t[:], hidT[b, ht * 128:(ht + 1) * 128, :])
                        hid_t[ht] = t
                        wqt = wsp.tile([128, HD], bf16, tag="w",
                                       name=f"wq_{bi}_{ht}")
                        nc.sync.dma_start(
                            wqt[:], wq[ht * 128:(ht + 1) * 128, :])
                        w_t[("q", ht)] = wqt

                    # Q then K: output-stationary chains per (head, sub)
                    for which, wsrc, bias_t, sc in (
                        ("q", wq, bq_t, scaling), ("k", wk, bk_t, 1.0)):
                        if which == "k":
                            for ht in hts:
                                wkt = wsp.tile([128, HD], bf16, tag="w",
                                               name=f"wk_{bi}_{ht}")
                                nc.sync.dma_start(
                                    wkt[:],
                                    wsrc[ht * 128:(ht + 1) * 128, :])
                                w_t[("k", ht)] = wkt
                        for head in range(hpc):
                            for sub in range(SB):
                                ps = flow_tile()
                                for i, ht in enumerate(hts):
                                    nc.tensor.matmul(
                                        ps[:],
                                        w_t[(which, ht)][:, head * 128:(head + 1) * 128],
                                        hid_t[ht][:, sub * 512:(sub + 1) * 512],
                                        start=(i == 0), stop=(i == CH - 1))
                                acc = qk_acc[which][head][:, sub * 512:(sub + 1) * 512]
                                if hc == 0:
                                    nc.vector.tensor_scalar(
                                        out=acc, in0=ps[:],
                                        scalar1=sc, scalar2=bias_t[:, head:head + 1],
                                        op0=ALU.mult, op1=ALU.add)
                                elif hc < N_CHUNKS - 1:
                                    nc.vector.scalar_tensor_tensor(
                                        out=acc, in0=ps[:],
                                        scalar=sc, in1=acc,
                                        op0=ALU.mult, op1=ALU.add)
                                else:
                                    fin = qk_final[which][head]
                                    nc.vector.scalar_tensor_tensor(
                                        out=fin[:, sub * 512:(sub + 1) * 512],
                                        in0=ps[:], scalar=sc, in1=acc,
                                        op0=ALU.mult, op1=ALU.add)

                    # V: output-stationary chains per s-tile
                    for ht in hts:
                        wvt = wsp.tile([128, HD], bf16, tag="w",
                                       name=f"wv_{bi}_{ht}")
                        nc.sync.dma_start(
                            wvt[:], wv[ht * 128:(ht + 1) * 128, :])
                        w_t[("v", ht)] = wvt
                    for st in range(ST):
                        ps = flow_tile(HD)
                        for i, ht in enumerate(hts):
                            nc.tensor.matmul(
                                ps[:],
                                hid_t[ht][:, st * 128:(st + 1) * 128],
                                w_t[("v", ht)][:],
                                start=(i == 0), stop=(i == CH - 1))
                        if hc == 0:
                            nc.vector.tensor_copy(v_acc[st][:], ps[:])
                        elif hc < N_CHUNKS - 1:
                            nc.vector.tensor_add(v_acc[st][:], ps[:], v_acc[st][:])
                        else:
                            nc.vector.tensor_add(v_final[st][:], ps[:], v_acc[st][:])

                # w_out resident load, once (overlaps batch-0 attention)
                if bi == 0:
                    for og in range(OG):
                        for f in range(hpc):
                            t = const.tile([128, 512], bf16, tag="wo",
                                           bufs=OG * hpc, name=f"wo_{og}_{f}")
                            nc.sync.dma_start(
                                t[:],
                                wo[f * 128:(f + 1) * 128,
                                   og * 512:(og + 1) * 512])
                            wo_t[(og, f)] = t

                # ====== attention (256-col q panels) + interleaved proj ======
                ctx_tiles = {h: [ctxp.tile([128, 512], bf16, tag="ctx",
                                           name=f"ctx_{bi}_{h}_{sb}")
                                 for sb in range(SB)] for h in range(hpc)}
                for qc in range(QC):
                    q0 = qc * 256
                    J = 2 * qc + 2
                    for hp in range(0, hpc, 2):
                        pair = [hp, hp + 1]
                        ps_ctx = held_tile()
                        ps_den = held_tile()
                        cs = {pair[0]: ps_ctx[:, 0:256],
                              pair[1]: ps_ctx[:, 256:512]}
                        ds = {pair[0]: ps_den[:, 0:256],
                              pair[1]: ps_den[:, 256:512]}
                        for j in range(J):
                            w0 = q0 - j * 128 + OFF
                            for h in pair:
                                qT = qk_final["q"][h]
                                kT = qk_final["k"][h]
                                ps_s = flow_tile(256)
                                nc.tensor.matmul(
                                    ps_s[:], kT[:, j * 128:(j + 1) * 128],
                                    qT[:, q0:q0 + 256],
                                    start=True, stop=True)
                                P = pp.tile([128, 256], bf16, tag="P")
                                nc.scalar.activation(P[:], ps_s[:], AF.Exp)
                                nc.vector.tensor_mul(P[:], P[:],
                                                     EAD[h][:, w0:w0 + 256])
                                # start=True clears has_written for the WHOLE
                                # bank, so only the first chain into a shared
                                # bank may use it; the partner head's first
                                # matmul lands on cleared bits and overwrites.
                                first = (j == 0 and h == pair[0])
                                nc.tensor.matmul(
                                    cs[h],
                                    v_final[j][:, h * 128:(h + 1) * 128],
                                    P[:], start=first, stop=(j == J - 1),
                                    skip_group_check=True)
                                nc.tensor.matmul(
                                    ds[h], ones[:], P[:],
                                    start=first, stop=(j == J - 1),
                                    skip_group_check=True)
                        for h in pair:
                            recip = rp.tile([128, 256], f32, tag="recip")
                            nc.vector.reciprocal_approx_fast(recip[:], ds[h])
                            half = (qc % 2) * 256
                            nc.vector.tensor_mul(
                                ctx_tiles[h][qc // 2][:, half:half + 256],
                                cs[h], recip[:])

                    # proj for seq block sb as soon as its ctx completes
                    if qc % 2 == 1:
                        sb = qc // 2
                        for og in range(OG):
                            for ot in range(4):
                                ps_o = flow_tile()
                                for f in range(hpc):
                                    nc.tensor.matmul(
                                        ps_o[:],
                                        wo_t[(og, f)][:, ot * 128:(ot + 1) * 128],
                                        ctx_tiles[f][sb][:],
                                        start=(f == 0), stop=(f == hpc - 1))
                                ost = osp.tile([128, 512], bf16, tag="ost")
                                nc.vector.tensor_copy(ost[:], ps_o[:])
                                r0 = og * 512 + ot * 128
                                c0 = b * S + sb * 512
                                nc.sync.dma_start(
                                    outp[r0:r0 + 128, c0:c0 + 512], ost[:])

    nc.compile()
    return nc


def _alibi_slopes(total_heads):
    closest = 2 ** math.floor(math.log2(total_heads))
    base = 2 ** (-(2 ** (-(math.log2(closest) - 3))))
    powers = np.arange(1, 1 + closest, dtype=np.float32)
    slopes = np.power(base, powers).astype(np.float32)
    if closest != total_heads:
        extra_base = 2 ** (-(2 ** (-(math.log2(2 * closest) - 3))))
        num_rem = min(closest, total_heads - closest)
        extra = np.arange(1, 1 + 2 * num_rem, 2, dtype=np.float32)
        slopes = np.concatenate(
            [slopes, np.power(extra_base, extra).astype(np.float32)])
    return slopes


_NC_CACHE = {}


def _get_nc():
    if "nc" not in _NC_CACHE:
        _NC_CACHE["nc"] = _build_nc()
    return _NC_CACHE["nc"]


def make_in_maps(hidden_states, w_qkv, b_qkv, w_out):
    """Build the 8 per-core input dicts."""
    import ml_dtypes
    bf = ml_dtypes.bfloat16
    scaling = np.float32(128 ** -0.5)
    hidT = np.ascontiguousarray(
        hidden_states.transpose(0, 2, 1)).astype(bf)
    slopes = _alibi_slopes(TOTAL_HEADS)
    nonce = np.zeros((1, 1), np.float32)
    in_maps = []
    for core in range(N_CORES):
        c0 = core * HD
        bq_v = b_qkv[c0:c0 + HD]
        bk_v = b_qkv[H + c0:H + c0 + HD]
        sl = slopes[core * HPC:(core + 1) * HPC]
        in_maps.append(dict(
            hidT=hidT,
            wq=np.ascontiguousarray(w_qkv[:, c0:c0 + HD]).astype(bf),
            wk=np.ascontiguousarray(w_qkv[:, H + c0:H + c0 + HD]).astype(bf),
            wv=np.ascontiguousarray(w_qkv[:, 2 * H + c0:2 * H + c0 + HD]).astype(bf),
            wo=np.ascontiguousarray(w_out[c0:c0 + HD, :]).astype(bf),
            bq=np.ascontiguousarray(
                (bq_v.reshape(HPC, 128).T * scaling).astype(np.float32)),
            bk=np.ascontiguousarray(bk_v.reshape(HPC, 128).T.astype(np.float32)),
            slp=np.ascontiguousarray(
                np.broadcast_to(sl[None, :], (128, HPC)).astype(np.float32)),
            nonce=nonce,
        ))
    return in_maps


def finish_output(partials, b_qkv, w_out, b_out):
    """Host-side all-reduce over cores + layout fix + bias."""
    total = np.zeros((H, B * S), dtype=np.float64)
    for p in partials:
        total += p.astype(np.float64)
    bias_vec = (b_qkv[2 * H:].astype(np.float64) @ w_out.astype(np.float64)
                + b_out.astype(np.float64))
    out = total.reshape(H, B, S).transpose(1, 2, 0) + bias_vec[None, None, :]
    return np.ascontiguousarray(out.astype(np.float32))


def kernel(hidden_states, w_qkv, b_qkv, w_out, b_out):
    hidden_states = np.asarray(hidden_states, dtype=np.float32)
    w_qkv = np.asarray(w_qkv, dtype=np.float32)
    b_qkv = np.asarray(b_qkv, dtype=np.float32)
    w_out = np.asarray(w_out, dtype=np.float32)
    b_out = np.asarray(b_out, dtype=np.float32)

    nc = _get_nc()
    in_maps = make_in_maps(hidden_states, w_qkv, b_qkv, w_out)
    res = run_bass_kernel_spmd(nc, in_maps, core_ids=list(range(N_CORES)))
    return finish_output([res.results[c]["outp"] for c in range(N_CORES)],
                         b_qkv, w_out, b_out)
